# revision 15
# baseline (speedup 1.0000x reference)
"""Trainium2 Bass kernel for nn_Deep_OSTTP_Model (deep tanh-LN recurrence with decayed trace).

Self-contained: takes FULL inputs, shards batch across 8 NeuronCores (pure data
parallel), runs a Bass/Tile kernel per core, gathers the full output.

Fast path exploits verified input structure:
  - Wz_w[i] == c_i * I  -> the recurrent matmul is a scalar multiply folded into
    LayerNorm algebra: LN(c*h + a) == (v - mean(v)) * rsqrt(var(v) + eps/c^2)
    with v = h + a/c (a = z @ Wx^T, pre-scaled by 1/c on the host).
  - all biases zero, ln_g == 1, ln_b == 0, R finite (the R projection is
    multiplied by 0.0 in the reference).
  - proj_in and Wx[0] are fused on the host: W0 = proj_in_w.T @ Wx_w[0].T / c0,
    so stage A directly produces layer-0's zWx (z0 itself is never needed).
If any structural check fails, a general numpy fallback computes the exact
reference math on the host.

Structure: the 4 row-chunks per core run as two independent pair-streams
({0,1} and {2,3}) so LN stats / Newton-rsqrt / tanh of one pair overlap the
other pair's work on different engines:
  - v = h + zw adds are pair-fused [128,2048] bf16 tensor ops: pair 0 on DVE
    (2x mode), pair 1 on GPSIMD (otherwise idle engine).
  - LN stats: bn_stats/bn_aggr on DVE; rsqrt via Newton iteration per pair.
  - tanh: ScalarE activation with per-row scale/bias.
  - decayed trace: chunks 0,1 accumulate on TensorE into persistent PSUM
    (start at t=0, stop at t=7); chunks 2,3 flush twice per layer through a
    transient PSUM tile with a DVE merge.
"""
import os
import numpy as np

L = 4
B = 4096
IN = 2048
H = 1024
OUT = 256
NSTEPS = 8
DECAY = 0.9
LN_EPS = 1e-5

NCORES = 8
BC = B // NCORES          # 512 rows per core
NCH = BC // 128           # 4 chunks of 128 rows
NPAIR = NCH // 2
KH = H // 128             # 8 contraction blocks over H
KIN = IN // 128           # 16 contraction blocks over IN
MAGIC = 0x5F3759DF

NEWTON_ITERS = int(os.environ.get("OSTTP_NEWTON_ITERS", "1"))
P1_ADD = os.environ.get("OSTTP_P1_ADD", "dve")   # engine for pair-1 adds

_cache = {}


def _build_program(c_per_layer):
    from contextlib import ExitStack
    import concourse.tile as tile
    from concourse import bacc, mybir

    F32 = mybir.dt.float32
    F32R = mybir.dt.float32r
    BF16 = mybir.dt.bfloat16
    U32 = mybir.dt.uint32
    I32 = mybir.dt.int32
    A = mybir.AluOpType
    TANH = mybir.ActivationFunctionType.Tanh

    nc = bacc.Bacc("TRN2", target_bir_lowering=False, debug=False)

    xT_d = nc.dram_tensor("xT", [IN, BC], BF16, kind="ExternalInput").ap()
    w0_d = nc.dram_tensor("w0", [IN, H], BF16, kind="ExternalInput").ap()
    wxT_d = nc.dram_tensor("wxT", [(L - 1) * H, H], BF16, kind="ExternalInput").ap()
    hd_d = nc.dram_tensor("hd", [128, KH * OUT], BF16, kind="ExternalInput").ap()
    idP16_d = nc.dram_tensor("idP16", [128, 128], BF16, kind="ExternalInput").ap()
    idR16_d = nc.dram_tensor("idR16", [128, NSTEPS * 128], BF16, kind="ExternalInput").ap()
    out_d = nc.dram_tensor("out", [BC, OUT], F32, kind="ExternalOutput").ap()

    with tile.TileContext(nc) as tc, ExitStack() as ctx:
        consts = ctx.enter_context(tc.tile_pool(name="consts", bufs=1))
        idP16 = consts.tile([128, 128], BF16)
        nc.sync.dma_start(idP16[:], idP16_d)
        idR16 = consts.tile([128, NSTEPS * 128], BF16)
        nc.sync.dma_start(idR16[:], idR16_d)
        hd_sb = consts.tile([128, KH * OUT], BF16)
        nc.sync.dma_start(hd_sb[:], hd_d)
        xT_sb = consts.tile([128, KIN * BC], BF16)
        for k in range(KIN):
            nc.sync.dma_start(xT_sb[:, k * BC:(k + 1) * BC],
                              xT_d[k * 128:(k + 1) * 128, :])

        # PSUM: vpp = v tiles for PE-add chunks 0,1 ; scr = everything else
        vpp = ctx.enter_context(tc.tile_pool(name="vpp", bufs=2, space="PSUM"))
        scr = ctx.enter_context(tc.tile_pool(name="scr", bufs=2, space="PSUM"))

        wxp = ctx.enter_context(tc.tile_pool(name="wxp", bufs=16))
        zwp = ctx.enter_context(tc.tile_pool(name="zwp", bufs=3))
        vp = ctx.enter_context(tc.tile_pool(name="vp", bufs=4))
        hp = ctx.enter_context(tc.tile_pool(name="hp", bufs=12))
        trsp = ctx.enter_context(tc.tile_pool(name="trsp", bufs=4))
        trTp = ctx.enter_context(tc.tile_pool(name="trTp", bufs=3))
        stp = ctx.enter_context(tc.tile_pool(name="stp", bufs=4))
        outp = ctx.enter_context(tc.tile_pool(name="outp", bufs=2))

        wx_sb = {}

        def load_wx(i):
            tiles = []
            for k in range(KH):
                t = wxp.tile([128, H], BF16, name=f"wx{i}_{k}", tag="wx", bufs=16)
                nc.sync.dma_start(
                    t[:], wxT_d[(i - 1) * H + k * 128:(i - 1) * H + (k + 1) * 128, :])
                tiles.append(t)
            wx_sb[i] = tiles

        # ---------------- Stage A: zw0 = x @ W0 (bf16, k-outer) --------------
        zw2 = [None] * NPAIR     # pair-fused [128, 2048] bf16 tiles
        with tc.tile_pool(name="w0p", bufs=KIN) as w0p:
            w0_tiles = []
            for k in range(KIN):
                w0k = w0p.tile([128, H], BF16, name=f"w0_{k}", tag="w0k", bufs=KIN)
                nc.sync.dma_start(w0k[:], w0_d[k * 128:(k + 1) * 128, :])
                w0_tiles.append(w0k)
            for p in range(NPAIR):
                zt = zwp.tile([128, 2 * H], BF16, name=f"zw0_{p}", tag="zw", bufs=3)
                for cl in range(2):
                    c = 2 * p + cl
                    ps = scr.tile([128, H], F32, name=f"zwA{c}", tag="scr", bufs=2)
                    for k in range(KIN):
                        for half in range(2):
                            nc.tensor.matmul(
                                ps[:, half * 512:(half + 1) * 512],
                                xT_sb[:, k * BC + c * 128:k * BC + (c + 1) * 128],
                                w0_tiles[k][:, half * 512:(half + 1) * 512],
                                start=(k == 0), stop=(k == KIN - 1))
                    nc.scalar.copy(zt[:, cl * H:(cl + 1) * H], ps[:])
                zw2[p] = zt

        load_wx(1)

        tr_sb = [None] * NCH     # bf16 sbuf traces (flush mode, all chunks)

        def transposes(c, src_bf16, note=""):
            """trT: [128, KH*128] bf16, block k = (src block k).T"""
            trT = trTp.tile([128, KH * 128], BF16, name=f"trT{note}_{c}", tag="trT")
            tp = scr.tile([128, 2 * H], BF16, name=f"tps{note}_{c}", tag="scr",
                          bufs=2)
            for k in range(KH):
                nc.tensor.transpose(tp[:, k * 128:(k + 1) * 128],
                                    src_bf16[:, k * 128:(k + 1) * 128], idP16[:])
            nc.scalar.copy(trT[:], tp[:, :KH * 128])
            return trT


        def emit_flush(i, p, t_lo, t_hi, hs):
            """trace partial for pair p: both chunks interleaved so idR_t
            stationary is shared; hs[t] are pair tiles [128, 2048]."""
            fps = [scr.tile([128, H], F32, name=f"fl{i}_{2*p+cl}_{t_lo}",
                            tag="scr", bufs=2) for cl in range(2)]
            for t in range(t_lo, t_hi + 1):
                for cl in range(2):
                    for half in range(2):
                        nc.tensor.matmul(
                            fps[cl][:, half * 512:(half + 1) * 512],
                            idR16[:, t * 128:(t + 1) * 128],
                            hs[t][:, cl * H + half * 512:cl * H + (half + 1) * 512],
                            start=(t == t_lo), stop=(t == t_hi))
            for cl in range(2):
                c = 2 * p + cl
                if t_lo == 0:
                    tr_c = trsp.tile([128, H], BF16, name=f"trf{i}_{c}", tag="trs")
                    nc.scalar.copy(tr_c[:], fps[cl][:])
                    tr_sb[c] = tr_c
                else:
                    nc.vector.tensor_tensor(tr_sb[c][:], tr_sb[c][:],
                                            fps[cl][:], op=A.add)

        def emit_boundary(i, p, trTs):
            """new pair zw tile for layer i from the pair's trT tiles."""
            zt = zwp.tile([128, 2 * H], BF16, name=f"zw{i}_{p}", tag="zw", bufs=3)
            for cl in range(2):
                zw_ps = scr.tile([128, H], F32, name=f"zwps{i}_{2*p+cl}",
                                 tag="scr", bufs=2)
                for k in range(KH):
                    for half in range(2):
                        nc.tensor.matmul(
                            zw_ps[:, half * 512:(half + 1) * 512],
                            trTs[cl][:, k * 128:(k + 1) * 128],
                            wx_sb[i][k][:, half * 512:(half + 1) * 512],
                            start=(k == 0), stop=(k == KH - 1))
                nc.scalar.copy(zt[:, cl * H:(cl + 1) * H], zw_ps[:])
            return zt

        def emit_pair_step(i, p, t, eps_i, h_prev, hs_kept):
            """one recurrence step for pair p; returns new h pair tile."""
            if t == 0:
                vsrc = zw2[p][:]
                vparts = [vsrc[:, :H], vsrc[:, H:]]
            elif p == 0:
                vparts = []
                for cl in range(2):
                    v_ps = vpp.tile([128, H], F32, name=f"v{i}_{cl}_{t}",
                                    tag="vps", bufs=2)
                    for half in range(2):
                        o = v_ps[:, half * 512:(half + 1) * 512]
                        nc.tensor.matmul(
                            o, idP16[:],
                            zw2[p][:, cl * H + half * 512:cl * H + (half + 1) * 512],
                            start=True, stop=False)
                        nc.tensor.matmul(
                            o, idP16[:],
                            h_prev[:, cl * H + half * 512:cl * H + (half + 1) * 512],
                            start=False, stop=True)
                    vparts.append(v_ps[:])
            else:
                v2 = vp.tile([128, 2 * H], BF16, name=f"v{i}_{p}_{t}", tag="v",
                             bufs=4)
                eng = nc.vector if P1_ADD == "dve" else nc.gpsimd
                eng.tensor_tensor(v2[:], h_prev[:], zw2[p][:], op=A.add)
                vsrc = v2[:]
                vparts = [vsrc[:, :H], vsrc[:, H:]]

            st2 = stp.tile([128, 24], F32, name=f"st{i}_{p}_{t}", tag="st")
            mv2 = stp.tile([128, 4], F32, name=f"mv{i}_{p}_{t}", tag="mv")
            y2 = stp.tile([128, 2], F32, name=f"y{i}_{p}_{t}", tag="y2")
            b2 = stp.tile([128, 2], F32, name=f"b{i}_{p}_{t}", tag="b2")
            nq2 = stp.tile([128, 2], F32, name=f"nq{i}_{p}_{t}", tag="nq")
            t2 = stp.tile([128, 2], F32, name=f"t{i}_{p}_{t}", tag="t2")
            for cl in range(2):
                for half in range(2):
                    nc.vector.bn_stats(
                        st2[:, cl * 12 + half * 6: cl * 12 + half * 6 + 6],
                        vparts[cl][:, half * 512:(half + 1) * 512])
                nc.vector.bn_aggr(mv2[:, cl * 2:(cl + 1) * 2],
                                  st2[:, cl * 12:(cl + 1) * 12]
                                  .rearrange("p (g s) -> p g s", s=6))
            mvw = mv2[:].rearrange("p (c s) -> p s c", s=2)
            mean2, var2 = mvw[:, 0, :], mvw[:, 1, :]
            nc.vector.tensor_scalar(nq2[:], var2, eps_i, -0.5, op0=A.add,
                                    op1=A.mult)
            nc.vector.tensor_scalar(y2[:].bitcast(U32), var2.bitcast(U32), 1,
                                    None, op0=A.logical_shift_right)
            nc.vector.tensor_scalar(y2[:].bitcast(I32), y2[:].bitcast(I32),
                                    -1, MAGIC, op0=A.mult, op1=A.add)
            for _ in range(NEWTON_ITERS):
                nc.vector.tensor_tensor(t2[:], y2[:], y2[:], op=A.mult)
                nc.vector.tensor_tensor(t2[:], t2[:], nq2[:], op=A.mult)
                nc.vector.scalar_tensor_tensor(y2[:], t2[:], 1.5, y2[:],
                                               op0=A.add, op1=A.mult)
            nc.vector.scalar_tensor_tensor(b2[:], mean2, -1.0, y2[:],
                                           op0=A.mult, op1=A.mult)

            h_new = hp.tile([128, 2 * H], BF16, name=f"h{i}_{p}_{t}", tag="h",
                            bufs=12)
            for cl in range(2):
                nc.scalar.activation(h_new[:, cl * H:(cl + 1) * H],
                                     vparts[cl], TANH,
                                     bias=b2[:, cl:cl + 1], scale=y2[:, cl:cl + 1])

            hs_kept[t] = h_new
            if t == 3:
                emit_flush(i, p, 0, 3, hs_kept)
            elif t == NSTEPS - 1:
                emit_flush(i, p, 4, NSTEPS - 1, hs_kept)
            return h_new

        for i in range(L):
            eps_i = LN_EPS / (c_per_layer[i] ** 2)
            if 1 <= i < L - 1:
                load_wx(i + 1)
            h_prev = [None] * NPAIR
            hs_kept = [dict(), dict()]
            for t in range(NSTEPS):
                h_prev[0] = emit_pair_step(i, 0, t, eps_i, h_prev[0], hs_kept[0])
                h_prev[1] = emit_pair_step(i, 1, t, eps_i, h_prev[1], hs_kept[1])
            # layer boundary / head, per pair
            for p in range(NPAIR):
                trTs = []
                for cl in range(2):
                    c = 2 * p + cl
                    trTs.append(transposes(c, tr_sb[c][:], note=f"L{i}"))
                if i < L - 1:
                    zw2[p] = emit_boundary(i + 1, p, trTs)
                else:
                    for cl in range(2):
                        c = 2 * p + cl
                        hd_ps = scr.tile([128, H], F32, name=f"hdps{c}",
                                         tag="scr", bufs=2)
                        for k in range(KH):
                            nc.tensor.matmul(hd_ps[:, :OUT],
                                             trTs[cl][:, k * 128:(k + 1) * 128],
                                             hd_sb[:, k * OUT:(k + 1) * OUT],
                                             start=(k == 0), stop=(k == KH - 1))
                        o_sb = outp.tile([128, OUT], F32, name=f"osb{c}", tag="o")
                        nc.scalar.copy(o_sb[:], hd_ps[:, :OUT])
                        nc.sync.dma_start(out_d[c * 128:(c + 1) * 128, :], o_sb[:])

    nc.compile()
    return nc


def _fallback_numpy(x, proj_in_w, proj_in_b, Wz_w, Wz_b, Wx_w, ln_g, ln_b, R,
                    head_w, head_b):
    x = x.astype(np.float32)
    z = x @ proj_in_w.T + proj_in_b
    for i in range(L):
        zWx = z @ Wx_w[i].T
        h = np.zeros_like(z)
        hs = []
        for _ in range(NSTEPS):
            u = h @ Wz_w[i].T + Wz_b[i] + zWx
            m = u.mean(axis=-1, keepdims=True)
            var = np.square(u - m).mean(axis=-1, keepdims=True)
            h = np.tanh((u - m) / np.sqrt(var + LN_EPS) * ln_g[i] + ln_b[i])
            hs.append(h)
        tr = np.zeros_like(z)
        for hh in hs:
            tr = DECAY * tr + hh
        dummy = np.ones((x.shape[0], OUT), dtype=x.dtype)
        proj = dummy @ R[i]
        z = tr + proj * 0.0
    return (z @ head_w.T + head_b).astype(np.float32)


def _check_structure(proj_in_b, Wz_w, Wz_b, ln_g, ln_b, R, head_b):
    cs = []
    eye = np.eye(H, dtype=np.float32)
    for i in range(L):
        c = float(Wz_w[i, 0, 0])
        if c <= 0 or not np.array_equal(Wz_w[i], c * eye):
            return None
        cs.append(c)
    if not (np.all(Wz_b == 0) and np.all(proj_in_b == 0) and np.all(head_b == 0)
            and np.all(ln_g == 1) and np.all(ln_b == 0) and np.all(np.isfinite(R))):
        return None
    return tuple(cs)


def _prep_in_maps(np_in, cs):
    import ml_dtypes
    bf16 = ml_dtypes.bfloat16

    x = np_in["x"].astype(np.float32, copy=False)
    pw = np_in["proj_in_w"].astype(np.float32)
    wx0 = np_in["Wx_w"][0].astype(np.float32)
    # fold proj_in and layer-0 Wx: zw0 = x @ (pw.T @ wx0.T) / c0
    w0 = np.ascontiguousarray((pw.T @ wx0.T) / cs[0]).astype(bf16)
    wxT = np.concatenate(
        [np.ascontiguousarray(np_in["Wx_w"][i].astype(np.float32).T) / cs[i]
         for i in range(1, L)], axis=0).astype(bf16)
    hd = np.ascontiguousarray(
        np_in["head_w"].astype(np.float32).T.reshape(KH, 128, OUT)
        .transpose(1, 0, 2).reshape(128, KH * OUT)).astype(bf16)
    idP16 = np.eye(128, dtype=np.float32).astype(bf16)
    idR16 = np.zeros((128, NSTEPS * 128), dtype=np.float32)
    for t in range(NSTEPS):
        idR16[:, t * 128:(t + 1) * 128] = (DECAY ** (NSTEPS - 1 - t)) * np.eye(
            128, dtype=np.float32)
    idR16 = idR16.astype(bf16)
    in_maps = []
    for s in range(NCORES):
        xT = np.ascontiguousarray(x[s * BC:(s + 1) * BC, :].T).astype(bf16)
        in_maps.append({"xT": xT, "w0": w0, "wxT": wxT, "hd": hd,
                       "idP16": idP16, "idR16": idR16})
    return in_maps


def kernel(**inputs):
    np_in = {k: np.asarray(v) for k, v in inputs.items()}
    cs = _check_structure(np_in["proj_in_b"], np_in["Wz_w"], np_in["Wz_b"],
                          np_in["ln_g"], np_in["ln_b"], np_in["R"], np_in["head_b"])
    if cs is None:
        return _fallback_numpy(**np_in)

    import concourse.bass_utils as bass_utils

    if cs not in _cache:
        _cache[cs] = _build_program(cs)
    nc = _cache[cs]
    in_maps = _prep_in_maps(np_in, cs)
    res = bass_utils.run_bass_kernel_spmd(nc, in_maps, core_ids=list(range(NCORES)))
    out = np.concatenate([res.results[s]["out"] for s in range(NCORES)], axis=0)
    return out.astype(np.float32)


def run_traced(np_in, trace_cores=None):
    """Run with NTFF tracing enabled; returns BassKernelResults."""
    import concourse.bass_utils as bass_utils
    np_in = {k: np.asarray(v) for k, v in np_in.items()}
    cs = _check_structure(np_in["proj_in_b"], np_in["Wz_w"], np_in["Wz_b"],
                          np_in["ln_g"], np_in["ln_b"], np_in["R"], np_in["head_b"])
    assert cs is not None
    if cs not in _cache:
        _cache[cs] = _build_program(cs)
    nc = _cache[cs]
    in_maps = _prep_in_maps(np_in, cs)
    return bass_utils.run_bass_kernel_spmd(
        nc, in_maps, core_ids=list(range(NCORES)), trace=True,
        trace_cores=trace_cores)


# revision 23
# speedup vs baseline: 1.1305x; 1.1305x over previous
"""Trainium2 Bass kernel for nn_Deep_OSTTP_Model (deep tanh-LN recurrence with decayed trace).

Self-contained: takes FULL inputs, shards batch across 8 NeuronCores (pure data
parallel), runs a Bass/Tile kernel per core, gathers the full output.

Fast path exploits verified input structure:
  - Wz_w[i] == c_i * I  -> the recurrent matmul is a scalar multiply folded into
    LayerNorm algebra: LN(c*h + a) == (v - mean(v)) * rsqrt(var(v) + eps/c^2)
    with v = h + a/c (a = z @ Wx^T, pre-scaled by 1/c on the host).
  - all biases zero, ln_g == 1, ln_b == 0, R finite (the R projection is
    multiplied by 0.0 in the reference).
  - proj_in and Wx[0] are fused on the host: W0 = proj_in_w.T @ Wx_w[0].T / c0,
    so stage A directly produces layer-0's zWx (z0 itself is never needed).
If any structural check fails, a general numpy fallback computes the exact
reference math on the host.

Structure: the 4 row-chunks per core run as two pair-streams ({0,1} and
{2,3}) whose work interleaves across engines:
  - v = h + zw adds: per-chunk bf16 DVE tensor ops (2x mode), each emitted
    right before that chunk's bn_stats so its stats start as soon as its own
    tanh from the previous step lands.
  - LN stats: bn_stats/bn_aggr on DVE; rsqrt via one Newton iteration per
    pair (magic-constant seed), batched on [128,2] tiles.
  - tanh: ScalarE activation with per-row scale/bias (all weights/biases of
    the LN are identity/zero, verified by the structure check).
  - decayed trace: all 4 chunks accumulate decay^(7-t)*h_t on TensorE into
    persistent PSUM slots via scaled-identity matmuls (start at t=0, stop at
    t=7); one accumulation group per chunk per layer.
  - layer boundary (trace copy, PE transposes, zw matmul) is emitted per pair
    immediately after that pair's last step so the other pair's remaining
    steps fill the gap.
"""
import os
import numpy as np

L = 4
B = 4096
IN = 2048
H = 1024
OUT = 256
NSTEPS = 8
DECAY = 0.9
LN_EPS = 1e-5

NCORES = 8
BC = B // NCORES          # 512 rows per core
NCH = BC // 128           # 4 chunks of 128 rows
NPAIR = NCH // 2
KH = H // 128             # 8 contraction blocks over H
KIN = IN // 128           # 16 contraction blocks over IN
MAGIC = 0x5F3759DF

NEWTON_ITERS = int(os.environ.get("OSTTP_NEWTON_ITERS", "1"))
P1_ADD = os.environ.get("OSTTP_P1_ADD", "dve")   # engine for pair-1 adds

_cache = {}


def _build_program(c_per_layer):
    from contextlib import ExitStack
    import concourse.tile as tile
    from concourse import bacc, mybir

    F32 = mybir.dt.float32
    F32R = mybir.dt.float32r
    BF16 = mybir.dt.bfloat16
    U32 = mybir.dt.uint32
    I32 = mybir.dt.int32
    A = mybir.AluOpType
    TANH = mybir.ActivationFunctionType.Tanh

    nc = bacc.Bacc("TRN2", target_bir_lowering=False, debug=False)

    xT_d = nc.dram_tensor("xT", [IN, BC], BF16, kind="ExternalInput").ap()
    w0_d = nc.dram_tensor("w0", [IN, H], BF16, kind="ExternalInput").ap()
    wxT_d = nc.dram_tensor("wxT", [(L - 1) * H, H], BF16, kind="ExternalInput").ap()
    hd_d = nc.dram_tensor("hd", [128, KH * OUT], BF16, kind="ExternalInput").ap()
    idP16_d = nc.dram_tensor("idP16", [128, 128], BF16, kind="ExternalInput").ap()
    idR16_d = nc.dram_tensor("idR16", [128, NSTEPS * 128], BF16, kind="ExternalInput").ap()
    out_d = nc.dram_tensor("out", [BC, OUT], F32, kind="ExternalOutput").ap()

    with tile.TileContext(nc) as tc, ExitStack() as ctx:
        consts = ctx.enter_context(tc.tile_pool(name="consts", bufs=1))
        idP16 = consts.tile([128, 128], BF16)
        nc.sync.dma_start(idP16[:], idP16_d)
        idR16 = consts.tile([128, NSTEPS * 128], BF16)
        nc.sync.dma_start(idR16[:], idR16_d)
        hd_sb = consts.tile([128, KH * OUT], BF16)
        nc.sync.dma_start(hd_sb[:], hd_d)
        xT_sb = consts.tile([128, KIN * BC], BF16)
        for k in range(KIN):
            nc.sync.dma_start(xT_sb[:, k * BC:(k + 1) * BC],
                              xT_d[k * 128:(k + 1) * 128, :])

        # PSUM: single 4-slot scratch pool (stage A, flushes, transposes,
        # boundary matmuls, head)
        scr = ctx.enter_context(tc.tile_pool(name="scr", bufs=4, space="PSUM"))

        wxp = ctx.enter_context(tc.tile_pool(name="wxp", bufs=16))
        zwp = ctx.enter_context(tc.tile_pool(name="zwp", bufs=3))
        vp = ctx.enter_context(tc.tile_pool(name="vp", bufs=4))
        hp = ctx.enter_context(tc.tile_pool(name="hp", bufs=12))
        trsp = ctx.enter_context(tc.tile_pool(name="trsp", bufs=4))
        trTp = ctx.enter_context(tc.tile_pool(name="trTp", bufs=3))
        stp = ctx.enter_context(tc.tile_pool(name="stp", bufs=4))
        outp = ctx.enter_context(tc.tile_pool(name="outp", bufs=2))

        wx_sb = {}

        def load_wx(i):
            tiles = []
            for k in range(KH):
                t = wxp.tile([128, H], BF16, name=f"wx{i}_{k}", tag="wx", bufs=16)
                nc.sync.dma_start(
                    t[:], wxT_d[(i - 1) * H + k * 128:(i - 1) * H + (k + 1) * 128, :])
                tiles.append(t)
            wx_sb[i] = tiles

        # ---------------- Stage A: zw0 = x @ W0 (bf16, k-outer) --------------
        zw2 = [None] * NPAIR     # pair-fused [128, 2048] bf16 tiles
        with tc.tile_pool(name="w0p", bufs=KIN) as w0p:
            w0_tiles = []
            for k in range(KIN):
                w0k = w0p.tile([128, H], BF16, name=f"w0_{k}", tag="w0k", bufs=KIN)
                nc.sync.dma_start(w0k[:], w0_d[k * 128:(k + 1) * 128, :])
                w0_tiles.append(w0k)
            for p in range(NPAIR):
                zt = zwp.tile([128, 2 * H], BF16, name=f"zw0_{p}", tag="zw", bufs=3)
                for cl in range(2):
                    c = 2 * p + cl
                    ps = scr.tile([128, H], F32, name=f"zwA{c}", tag="scr", bufs=4)
                    for k in range(KIN):
                        for half in range(2):
                            nc.tensor.matmul(
                                ps[:, half * 512:(half + 1) * 512],
                                xT_sb[:, k * BC + c * 128:k * BC + (c + 1) * 128],
                                w0_tiles[k][:, half * 512:(half + 1) * 512],
                                start=(k == 0), stop=(k == KIN - 1))
                    nc.scalar.copy(zt[:, cl * H:(cl + 1) * H], ps[:])
                zw2[p] = zt

        load_wx(1)

        tr_sb = [None] * NCH     # bf16 sbuf traces (flush mode, all chunks)

        def transposes(c, src_bf16, note=""):
            """trT: [128, KH*128] bf16, block k = (src block k).T"""
            trT = trTp.tile([128, KH * 128], BF16, name=f"trT{note}_{c}", tag="trT")
            tp = scr.tile([128, 2 * H], BF16, name=f"tps{note}_{c}", tag="scr",
                          bufs=4)
            for k in range(KH):
                nc.tensor.transpose(tp[:, k * 128:(k + 1) * 128],
                                    src_bf16[:, k * 128:(k + 1) * 128], idP16[:])
            nc.scalar.copy(trT[:], tp[:, :KH * 128])
            return trT


        def emit_flush(i, p, t_lo, t_hi, hs):
            """trace partial for pair p: both chunks interleaved so idR_t
            stationary is shared; hs[t] are pair tiles [128, 2048]."""
            fps = [scr.tile([128, H], F32, name=f"fl{i}_{2*p+cl}_{t_lo}",
                            tag="scr", bufs=4) for cl in range(2)]
            for t in range(t_lo, t_hi + 1):
                for cl in range(2):
                    for half in range(2):
                        nc.tensor.matmul(
                            fps[cl][:, half * 512:(half + 1) * 512],
                            idR16[:, t * 128:(t + 1) * 128],
                            hs[t][:, cl * H + half * 512:cl * H + (half + 1) * 512],
                            start=(t == t_lo), stop=(t == t_hi))
            for cl in range(2):
                c = 2 * p + cl
                if t_lo == 0:
                    tr_c = trsp.tile([128, H], BF16, name=f"trf{i}_{c}", tag="trs")
                    nc.scalar.copy(tr_c[:], fps[cl][:])
                    tr_sb[c] = tr_c
                else:
                    nc.vector.tensor_tensor(tr_sb[c][:], tr_sb[c][:],
                                            fps[cl][:], op=A.add)

        def emit_boundary(i, p, trTs):
            """new pair zw tile for layer i from the pair's trT tiles."""
            zt = zwp.tile([128, 2 * H], BF16, name=f"zw{i}_{p}", tag="zw", bufs=3)
            for cl in range(2):
                zw_ps = scr.tile([128, H], F32, name=f"zwps{i}_{2*p+cl}",
                                 tag="scr", bufs=4)
                for k in range(KH):
                    for half in range(2):
                        nc.tensor.matmul(
                            zw_ps[:, half * 512:(half + 1) * 512],
                            trTs[cl][:, k * 128:(k + 1) * 128],
                            wx_sb[i][k][:, half * 512:(half + 1) * 512],
                            start=(k == 0), stop=(k == KH - 1))
                nc.scalar.copy(zt[:, cl * H:(cl + 1) * H], zw_ps[:])
            return zt

        def emit_pair_stats(i, p, t, h_prev):
            """add + bn stats for pair p; returns (vparts, mv2 view)."""
            if t == 0:
                vsrc = zw2[p][:]
                vparts = [vsrc[:, :H], vsrc[:, H:]]
            else:
                v2 = vp.tile([128, 2 * H], BF16, name=f"v{i}_{p}_{t}", tag="v",
                             bufs=4)
                vparts = [v2[:, :H], v2[:, H:]]

            st2 = stp.tile([128, 24], F32, name=f"st{i}_{p}_{t}", tag=f"st{p}")
            mv2 = stp.tile([128, 4], F32, name=f"mv{i}_{p}_{t}", tag=f"mv{p}")
            for cl in range(2):
                if t > 0:
                    nc.vector.tensor_tensor(vparts[cl],
                                            h_prev[:, cl * H:(cl + 1) * H],
                                            zw2[p][:, cl * H:(cl + 1) * H],
                                            op=A.add)
                for half in range(2):
                    nc.vector.bn_stats(
                        st2[:, cl * 12 + half * 6: cl * 12 + half * 6 + 6],
                        vparts[cl][:, half * 512:(half + 1) * 512])
                nc.vector.bn_aggr(mv2[:, cl * 2:(cl + 1) * 2],
                                  st2[:, cl * 12:(cl + 1) * 12]
                                  .rearrange("p (g s) -> p g s", s=6))
            return vparts, mv2

        def emit_newton(i, p, t, eps_i, mv2):
            """pair Newton-rsqrt; y = rsqrt(var+eps), b = -mean*y."""
            y2 = stp.tile([128, 2], F32, name=f"y{i}_{p}_{t}", tag=f"y{p}")
            b2 = stp.tile([128, 2], F32, name=f"b{i}_{p}_{t}", tag=f"b{p}")
            nq2 = stp.tile([128, 2], F32, name=f"nq{i}_{p}_{t}", tag=f"nq{p}")
            t2 = stp.tile([128, 2], F32, name=f"t{i}_{p}_{t}", tag=f"t{p}")
            mvw = mv2[:].rearrange("p (c s) -> p s c", s=2)
            mean2, var2 = mvw[:, 0, :], mvw[:, 1, :]
            nc.vector.tensor_scalar(nq2[:], var2, eps_i, -0.5, op0=A.add,
                                    op1=A.mult)
            nc.vector.tensor_scalar(y2[:].bitcast(U32), var2.bitcast(U32), 1,
                                    None, op0=A.logical_shift_right)
            nc.vector.tensor_scalar(y2[:].bitcast(I32), y2[:].bitcast(I32),
                                    -1, MAGIC, op0=A.mult, op1=A.add)
            for _ in range(NEWTON_ITERS):
                nc.vector.tensor_tensor(t2[:], y2[:], y2[:], op=A.mult)
                nc.vector.tensor_tensor(t2[:], t2[:], nq2[:], op=A.mult)
                nc.vector.scalar_tensor_tensor(y2[:], t2[:], 1.5, y2[:],
                                               op0=A.add, op1=A.mult)
            nc.vector.scalar_tensor_tensor(b2[:], mean2, -1.0, y2[:],
                                           op0=A.mult, op1=A.mult)
            return y2, b2

        def emit_pair_tail(i, p, t, vparts, y2, b2):
            """tanh + trace accumulation for pair p."""
            h_new = hp.tile([128, 2 * H], BF16, name=f"h{i}_{p}_{t}", tag="h",
                            bufs=6)
            for cl in range(2):
                nc.scalar.activation(h_new[:, cl * H:(cl + 1) * H],
                                     vparts[cl], TANH,
                                     bias=b2[:, cl:cl + 1], scale=y2[:, cl:cl + 1])
            for cl in range(2):
                c = 2 * p + cl
                if t == 0:
                    tr_ps[c] = scr.tile([128, H], F32, name=f"trp{i}_{c}",
                                        tag="scr", bufs=4)
                for half in range(2):
                    nc.tensor.matmul(
                        tr_ps[c][:, half * 512:(half + 1) * 512],
                        idR16[:, t * 128:(t + 1) * 128],
                        h_new[:, cl * H + half * 512:cl * H + (half + 1) * 512],
                        start=(t == 0), stop=(t == NSTEPS - 1))
            return h_new

        for i in range(L):
            eps_i = LN_EPS / (c_per_layer[i] ** 2)
            if 1 <= i < L - 1:
                load_wx(i + 1)
            h_prev = [None] * NPAIR
            hs_kept = [dict(), dict()]
            for t in range(NSTEPS):
                h_prev[0] = emit_pair_step(i, 0, t, eps_i, h_prev[0], hs_kept[0])
                h_prev[1] = emit_pair_step(i, 1, t, eps_i, h_prev[1], hs_kept[1])
            # layer boundary / head, per pair
            for p in range(NPAIR):
                trTs = []
                for cl in range(2):
                    c = 2 * p + cl
                    trTs.append(transposes(c, tr_sb[c][:], note=f"L{i}"))
                if i < L - 1:
                    zw2[p] = emit_boundary(i + 1, p, trTs)
                else:
                    for cl in range(2):
                        c = 2 * p + cl
                        hd_ps = scr.tile([128, H], F32, name=f"hdps{c}",
                                         tag="scr", bufs=4)
                        for k in range(KH):
                            nc.tensor.matmul(hd_ps[:, :OUT],
                                             trTs[cl][:, k * 128:(k + 1) * 128],
                                             hd_sb[:, k * OUT:(k + 1) * OUT],
                                             start=(k == 0), stop=(k == KH - 1))
                        o_sb = outp.tile([128, OUT], F32, name=f"osb{c}", tag="o")
                        nc.scalar.copy(o_sb[:], hd_ps[:, :OUT])
                        nc.sync.dma_start(out_d[c * 128:(c + 1) * 128, :], o_sb[:])

    nc.compile()
    return nc


def _fallback_numpy(x, proj_in_w, proj_in_b, Wz_w, Wz_b, Wx_w, ln_g, ln_b, R,
                    head_w, head_b):
    x = x.astype(np.float32)
    z = x @ proj_in_w.T + proj_in_b
    for i in range(L):
        zWx = z @ Wx_w[i].T
        h = np.zeros_like(z)
        hs = []
        for _ in range(NSTEPS):
            u = h @ Wz_w[i].T + Wz_b[i] + zWx
            m = u.mean(axis=-1, keepdims=True)
            var = np.square(u - m).mean(axis=-1, keepdims=True)
            h = np.tanh((u - m) / np.sqrt(var + LN_EPS) * ln_g[i] + ln_b[i])
            hs.append(h)
        tr = np.zeros_like(z)
        for hh in hs:
            tr = DECAY * tr + hh
        dummy = np.ones((x.shape[0], OUT), dtype=x.dtype)
        proj = dummy @ R[i]
        z = tr + proj * 0.0
    return (z @ head_w.T + head_b).astype(np.float32)


def _check_structure(proj_in_b, Wz_w, Wz_b, ln_g, ln_b, R, head_b):
    cs = []
    eye = np.eye(H, dtype=np.float32)
    for i in range(L):
        c = float(Wz_w[i, 0, 0])
        if c <= 0 or not np.array_equal(Wz_w[i], c * eye):
            return None
        cs.append(c)
    if not (np.all(Wz_b == 0) and np.all(proj_in_b == 0) and np.all(head_b == 0)
            and np.all(ln_g == 1) and np.all(ln_b == 0) and np.all(np.isfinite(R))):
        return None
    return tuple(cs)


def _prep_in_maps(np_in, cs):
    import ml_dtypes
    bf16 = ml_dtypes.bfloat16

    x = np_in["x"].astype(np.float32, copy=False)
    pw = np_in["proj_in_w"].astype(np.float32)
    wx0 = np_in["Wx_w"][0].astype(np.float32)
    # fold proj_in and layer-0 Wx: zw0 = x @ (pw.T @ wx0.T) / c0
    w0 = np.ascontiguousarray((pw.T @ wx0.T) / cs[0]).astype(bf16)
    wxT = np.concatenate(
        [np.ascontiguousarray(np_in["Wx_w"][i].astype(np.float32).T) / cs[i]
         for i in range(1, L)], axis=0).astype(bf16)
    hd = np.ascontiguousarray(
        np_in["head_w"].astype(np.float32).T.reshape(KH, 128, OUT)
        .transpose(1, 0, 2).reshape(128, KH * OUT)).astype(bf16)
    idP16 = np.eye(128, dtype=np.float32).astype(bf16)
    idR16 = np.zeros((128, NSTEPS * 128), dtype=np.float32)
    for t in range(NSTEPS):
        idR16[:, t * 128:(t + 1) * 128] = (DECAY ** (NSTEPS - 1 - t)) * np.eye(
            128, dtype=np.float32)
    idR16 = idR16.astype(bf16)
    in_maps = []
    for s in range(NCORES):
        xT = np.ascontiguousarray(x[s * BC:(s + 1) * BC, :].T).astype(bf16)
        in_maps.append({"xT": xT, "w0": w0, "wxT": wxT, "hd": hd,
                       "idP16": idP16, "idR16": idR16})
    return in_maps


def kernel(**inputs):
    np_in = {k: np.asarray(v) for k, v in inputs.items()}
    cs = _check_structure(np_in["proj_in_b"], np_in["Wz_w"], np_in["Wz_b"],
                          np_in["ln_g"], np_in["ln_b"], np_in["R"], np_in["head_b"])
    if cs is None:
        return _fallback_numpy(**np_in)

    import concourse.bass_utils as bass_utils

    if cs not in _cache:
        _cache[cs] = _build_program(cs)
    nc = _cache[cs]
    in_maps = _prep_in_maps(np_in, cs)
    res = bass_utils.run_bass_kernel_spmd(nc, in_maps, core_ids=list(range(NCORES)))
    out = np.concatenate([res.results[s]["out"] for s in range(NCORES)], axis=0)
    return out.astype(np.float32)


def run_traced(np_in, trace_cores=None):
    """Run with NTFF tracing enabled; returns BassKernelResults."""
    import concourse.bass_utils as bass_utils
    np_in = {k: np.asarray(v) for k, v in np_in.items()}
    cs = _check_structure(np_in["proj_in_b"], np_in["Wz_w"], np_in["Wz_b"],
                          np_in["ln_g"], np_in["ln_b"], np_in["R"], np_in["head_b"])
    assert cs is not None
    if cs not in _cache:
        _cache[cs] = _build_program(cs)
    nc = _cache[cs]
    in_maps = _prep_in_maps(np_in, cs)
    return bass_utils.run_bass_kernel_spmd(
        nc, in_maps, core_ids=list(range(NCORES)), trace=True,
        trace_cores=trace_cores)
